# revision 37
# baseline (speedup 1.0000x reference)
"""Mimi-style GQA attention (RoPE + 250-wide sliding causal window) on 8 TRN2 NeuronCores.

Sharding (per spec hint): core c handles batch b=c//4 and KV-head group g=c%4
(4 query heads + 1 KV head). Wq/Wk/Wv column-sharded, Wo row-sharded along the
head dim; each core emits a partial [S, D] fp16 output; host sums the 4
partials per batch in fp32.

Per-core pipeline (all matmul inputs fp16, PSUM accumulation fp32):
  1. qT projections plus a PACKED kv projection (K in PSUM rows 0:64, V in
     rows 64:128) from host-pre-transposed hsT, dt iterated in DMA-arrival
     order across the two load queues. RoPE applied as raw*cos +
     (PERM @ raw)*sin_signed with the PERM matmul on the tensor engine; the
     roped K is partition-shift-DMA'd up to rows 64:128 of kdup so score
     matmuls row-pack two heads; V is transposed per k-tile straight out of
     rows 64:128 (identity block lives at partitions 64:128).
  2. Banded flash attention in scoresT [k, q] orientation: per 128-wide k-tile
     only the 384-wide in-window q-range is computed; exp (with fused 1/8
     scale) on ScalarE; band mask multiply on VectorE over only the two
     136-wide uncertain column bands; PV accumulated per 512-wide q-strip into
     [65, 2, 512] PSUM tiles where row 64 (a ones column in the V stationary)
     accumulates the softmax denominators for free.  No PSUM pre-seeding: the
     diagonal piece clears each bank with start=True and the piece order keeps
     every matmul range uniformly fresh-or-written (kt=4s+1 split at col 384).
  3. PV PSUM is drained to SBUF fp16 immediately (frees the bank for the next
     strip); denominators (row 64) go through one merged per-strip
     reshape-DMA -> f32 reciprocal -> broadcast chain; normalize multiplies
     are cheap fp16 SBUF ops; row-sharded Wo projection emits fp16 partials,
     each store split across the SP and Pool DMA queues.  Lowest-priority
     "warmkeeper" matmuls run whenever the tensor engine would idle, keeping
     the HAM p-state warm through normalize-chain stalls.
"""
import os
import sys

for _p in ("/opt/trn_rl_repo", "/root/.axon_site/_ro/trn_rl_repo"):
    if os.path.isdir(_p) and _p not in sys.path:
        sys.path.append(_p)

import numpy as np
import concourse.bass as bass
import concourse.mybir as mybir
import concourse.tile as tile
from concourse import bacc
from concourse.bass_utils import run_bass_kernel_spmd
from concourse.masks import make_identity

F32 = mybir.dt.float32
F16 = mybir.dt.float16
AF = mybir.ActivationFunctionType
OP = mybir.AluOpType

B, S, D = 2, 2048, 1024
H, HK, HD = 16, 4, 64
WINDOW = 250
SCALE = 1.0 / np.sqrt(HD)
THETA = 10000.0
NKT = S // 128          # 16 k-tiles
NST = S // 512          # 4 q-strips
WIN = 384               # padded per-k-tile q-window


def _pv_pieces(s):
    """PV pieces for q-strip s: list of (kt, c0, c1) window-column ranges.

    Ordered so the first piece (the diagonal kt=4s, covering strip cols
    0:384 with start=True) clears the PSUM bank, and every later piece's
    column range is uniformly already-written (accumulate) or uniformly
    fresh (overwrite via cleared has_written).  kt=4s+1 is split at the
    384 boundary to keep that invariant.
    """
    ranges = {}
    for kt in range(max(0, 4 * s - 2), min(NKT - 1, 4 * s + 3) + 1):
        j0 = 128 * kt
        w = min(WIN, S - j0)
        c_lo = max(0, 512 * s - j0)
        c_hi = min(w, 512 * (s + 1) - j0)
        if c_lo < c_hi:
            ranges[kt] = (c_lo, c_hi)
    out = []
    for kt in (4 * s, 4 * s + 1, 4 * s + 2, 4 * s + 3, 4 * s - 1, 4 * s - 2):
        if kt not in ranges:
            continue
        c0, c1 = ranges[kt]
        if kt == 4 * s + 1 and c0 < 256 < c1:
            out.append((kt, c0, 256))
            out.append((kt, 256, c1))
        else:
            out.append((kt, c0, c1))
    return out


def _build():
    nc = bacc.Bacc(None, target_bir_lowering=False)

    hsT = nc.declare_dram_parameter("hsT", [8, 128, S], F16, isOutput=False)
    wq = nc.declare_dram_parameter("wqT", [8, 128, 256], F16, isOutput=False)
    wkv = nc.declare_dram_parameter("wkvT", [8, 128, 128], F16, isOutput=False)
    wo = nc.declare_dram_parameter("woT", [2, 128, D], F16, isOutput=False)
    cosd = nc.declare_dram_parameter("cos2", [128, S], F16, isOutput=False)
    sind = nc.declare_dram_parameter("sinS2", [128, S], F16, isOutput=False)
    maskd = nc.declare_dram_parameter("bandmask", [128, WIN], F16, isOutput=False)
    permd = nc.declare_dram_parameter("permT", [128, 128], F16, isOutput=False)
    o_part = nc.declare_dram_parameter("o_part", [S, D], F16, isOutput=True)

    with tile.TileContext(nc) as tc:
        with (
            tc.tile_pool(name="persist", bufs=1) as pp,
            tc.tile_pool(name="work", bufs=3) as wk_pool,
            tc.tile_pool(name="expm", bufs=30) as ep,
            tc.tile_pool(name="norm", bufs=2) as npool,
            tc.tile_pool(name="ost", bufs=4) as opool,
        ):
            # ---- persistent loads, split across the SP and Pool DMA queues,
            # ordered by first use so early matmuls start ASAP ----
            ht = [pp.tile([128, S], F16, tag=f"ht{dt}", name=f"ht{dt}")
                  for dt in range(8)]
            wq_sb = pp.tile([128, 8, 256], F16, tag="wq")
            nc.sync.dma_start(out=wq_sb[:, :, 0:128],
                              in_=wq.rearrange("a p c -> p a c")[:, :, 0:128])
            nc.sync.dma_start(out=ht[0][:, 0:1024], in_=hsT[0][:, 0:1024])
            nc.sync.dma_start(out=ht[0][:, 1024:2048], in_=hsT[0][:, 1024:2048])
            for dt in range(1, 4):
                nc.sync.dma_start(out=ht[dt], in_=hsT[dt])
            wkv_sb = pp.tile([128, 8, 128], F16, tag="wkv")
            nc.gpsimd.dma_start(out=wkv_sb, in_=wkv.rearrange("a p c -> p a c"))
            for dt in range(4, 8):
                nc.gpsimd.dma_start(out=ht[dt], in_=hsT[dt])
            cos_sb = pp.tile([128, S], F16, tag="cos")
            sin_sb = pp.tile([128, S], F16, tag="sin")
            nc.gpsimd.dma_start(out=cos_sb, in_=cosd[:, :])
            nc.gpsimd.dma_start(out=sin_sb, in_=sind[:, :])
            perm_sb = pp.tile([128, 128], F16, tag="perm")
            nc.gpsimd.dma_start(out=perm_sb, in_=permd[:, :])
            mask_sb = pp.tile([128, WIN], F16, tag="mask")
            nc.gpsimd.dma_start(out=mask_sb, in_=maskd[:, :])
            wo_sb = pp.tile([128, 2, D], F16, tag="wo")
            nc.gpsimd.dma_start(out=wo_sb, in_=wo.rearrange("a p c -> p a c"))
            nc.sync.dma_start(out=wq_sb[:, :, 128:256],
                              in_=wq.rearrange("a p c -> p a c")[:, :, 128:256])
            ident = pp.tile([128, 64], F16, tag="ident")
            make_identity(nc, ident[64:128, :])
            wsrc = pp.tile([128, WIN], F16, tag="wsrc")
            nc.gpsimd.memset(wsrc, 0.0)

            qT = [pp.tile([128, S], F16, tag=f"qT{m}", name=f"qT{m}") for m in range(2)]
            kdup = pp.tile([128, S], F16, tag="kdup")
            vhi = pp.tile([128, S], F16, tag="vhi")
            vaug = [pp.tile([128, 65], F16, tag=f"vaug{kt}", name=f"vaug{kt}") for kt in range(NKT)]
            aT = [pp.tile([128, S], F16, tag=f"aT{m}", name=f"aT{m}") for m in range(2)]
            stag = [pp.tile([64, S], F16, tag=f"stag{m}", name=f"stag{m}") for m in range(2)]

            # ---- phase 1: projections + rope ----
            with (
                tc.tile_pool(name="ps1", bufs=6, space="PSUM") as ps1,
                tc.tile_pool(name="ps1b", bufs=2, space="PSUM") as ps1b,
            ):
                def proj_group(lhs_of_dt, consume, gname):
                    """One projection over all 4 strips, dt-outer so each
                    stationary is loaded once and reused across strips."""
                    pstiles = [ps1.tile([128, 512], F32, tag="proj",
                                        name=f"pj{gname}_{_s}") for _s in range(NST)]
                    dts = (0, 4, 1, 5, 2, 6, 3, 7)
                    for i, dt in enumerate(dts):
                        for strip in range(NST):
                            nc.tensor.matmul(pstiles[strip], lhs_of_dt(dt),
                                             ht[dt][:, bass.ts(strip, 512)],
                                             start=(i == 0), stop=(i == 7))
                    for strip in range(NST):
                        consume(pstiles[strip], strip)

                def q_consume(m):
                    def go(ps, strip):
                        sl = bass.ts(strip, 512)
                        raw = wk_pool.tile([128, 512], F16, tag="rawq")
                        nc.scalar.copy(raw, ps)
                        rot = ps1b.tile([128, 512], F32, tag="rot", name="rotq")
                        nc.tensor.matmul(rot, perm_sb, raw, start=True, stop=True)
                        t1 = wk_pool.tile([128, 512], F16, tag="t1q")
                        nc.vector.tensor_tensor(out=t1, in0=rot, in1=sin_sb[:, sl],
                                                op=OP.mult)
                        t2 = wk_pool.tile([128, 512], F16, tag="t2q")
                        nc.gpsimd.tensor_tensor(out=t2, in0=raw, in1=cos_sb[:, sl],
                                                op=OP.mult)
                        nc.vector.tensor_tensor(out=qT[m][:, sl], in0=t1, in1=t2,
                                                op=OP.add)
                    return go

                def kv_consume(ps, strip):
                    sl = bass.ts(strip, 512)
                    # K sits in PSUM rows 0:64 — rope it exactly like baseline
                    kraw = wk_pool.tile([64, 512], F16, tag="kraw")
                    nc.scalar.copy(kraw, ps[0:64, :])
                    rot = ps1b.tile([64, 512], F32, tag="rot", name="rotk")
                    nc.tensor.matmul(rot, perm_sb[0:64, 0:64], kraw,
                                     start=True, stop=True)
                    t1 = wk_pool.tile([64, 512], F16, tag="t1k")
                    nc.vector.tensor_tensor(out=t1, in0=rot, in1=sin_sb[0:64, sl],
                                            op=OP.mult)
                    t2 = wk_pool.tile([64, 512], F16, tag="t2k")
                    nc.gpsimd.tensor_tensor(out=t2, in0=kraw, in1=cos_sb[0:64, sl],
                                            op=OP.mult)
                    nc.vector.tensor_tensor(out=kdup[0:64, sl], in0=t1, in1=t2,
                                            op=OP.add)
                    nc.sync.dma_start(out=kdup[64:128, sl], in_=kdup[0:64, sl])
                    # V sits in rows 64:128; transposes read it there directly
                    nc.vector.tensor_copy(vhi[64:128, sl], ps[64:128, :])

                proj_group(lambda dt: wq_sb[:, dt, bass.ts(0, 128)],
                           q_consume(0), "q0")
                proj_group(lambda dt: wkv_sb[:, dt], kv_consume, "kv")
                proj_group(lambda dt: wq_sb[:, dt, bass.ts(1, 128)],
                           q_consume(1), "q1")

                for kt in range(NKT):
                    pvt = ps1b.tile([128, 64], F16, tag="rot", name=f"vtr{kt}")
                    nc.tensor.transpose(pvt, vhi[64:128, bass.ts(kt, 128)],
                                        ident[64:128, :])
                    nc.vector.tensor_copy(vaug[kt][:, 0:64], pvt)
                    nc.vector.memset(vaug[kt][:, 64:65], 1.0)

            # ---- phases 2+3: banded attention + output projection ----
            with (
                tc.tile_pool(name="ps2", bufs=2, space="PSUM") as ps2,
                tc.tile_pool(name="pspv", bufs=2, space="PSUM") as pspv,
                tc.tile_pool(name="ps3", bufs=2, space="PSUM") as ps3,
            ):
                wk_count = [0]

                def warmkeepers(n):
                    for _ in range(n):
                        i = wk_count[0]
                        wk_count[0] += 1
                        wps = ps2.tile([128, WIN], F32, tag="sc", name=f"warm{i}")
                        nc.tensor.matmul(wps, wsrc[:, 0:128], wsrc,
                                         start=True, stop=True)

                expm = {}
                for s in range(NST):
                    # fresh expT tiles for this strip's new k-tiles
                    kts = range(max(0, 4 * s - 2), min(NKT - 1, 4 * s + 3) + 1)
                    for kt in kts:
                        if (kt, 0) in expm:
                            continue
                        j0 = 128 * kt
                        w = min(WIN, S - j0)
                        for h in range(4):
                            m, half = h // 2, (h % 2) * 64
                            pss = ps2.tile([128, WIN], F32, tag="sc")
                            nc.tensor.matmul(
                                pss[:, 0:w],
                                kdup[half:half + 64, bass.ts(kt, 128)],
                                qT[m][half:half + 64, j0:j0 + w],
                                start=True, stop=True)
                            et = ep.tile([128, WIN], F16, tag="e")
                            nc.scalar.activation(et[:, 0:w], pss[:, 0:w], AF.Exp,
                                                 scale=float(SCALE))
                            if w == WIN:
                                # only cols [0,136) and [248,384) are mask-
                                # uncertain; the middle is always in-band
                                blk = bass.AP(tensor=et.tensor, offset=et.offset,
                                              ap=[list(et.ap[0]), [248, 2], [1, 136]])
                                mblk = bass.AP(tensor=mask_sb.tensor, offset=mask_sb.offset,
                                               ap=[list(mask_sb.ap[0]), [248, 2], [1, 136]])
                                nc.vector.tensor_tensor(out=blk, in0=blk, in1=mblk,
                                                        op=OP.mult)
                            else:
                                nc.vector.tensor_tensor(out=et[:, 0:w], in0=et[:, 0:w],
                                                        in1=mask_sb[:, 0:w], op=OP.mult)
                            expm[(kt, h)] = et

                    pieces = _pv_pieces(s)
                    pvs = []
                    for hp in range(2):
                        pvp = pspv.tile([65, 2, 512], F32, tag="pv",
                                        name=f"pv{hp}_{s}")
                        for i, (kt, c0, c1) in enumerate(pieces):
                            base = 128 * kt + c0 - 512 * s
                            last = i == len(pieces) - 1
                            for hh in range(2):
                                nc.tensor.matmul(pvp[:, hh, base:base + (c1 - c0)],
                                                 vaug[kt],
                                                 expm[(kt, 2 * hp + hh)][:, c0:c1],
                                                 start=(i == 0), stop=last)
                        # drain to SBUF fp16 right away to free the PSUM bank
                        pv16 = npool.tile([65, 1024], F16, tag=f"pv16_{hp}",
                                          name=f"pv16_{hp}_{s}")
                        if hp == 0:
                            nc.scalar.copy(pv16, pvp)
                        else:
                            nc.vector.tensor_copy(pv16, pvp)
                        pvs.append(pv16)

                    # softmax denominators -> broadcast reciprocals.
                    # Early strips: reshape-DMA + f32 reciprocal (cheap on
                    # engines, long latency, hidden under later strips).
                    # Tail strips: recip = exp(-ln(den)) on the [1,1024] rows
                    # (same ACT table set as the score exps) — short latency
                    # chain since nothing hides it at the end of the kernel.
                    rsp = npool.tile([16, 128], F32, tag="rsp")
                    nc.gpsimd.dma_start(out=rsp[0:8, :], in_=pvs[0][64:65, :])
                    nc.gpsimd.dma_start(out=rsp[8:16, :], in_=pvs[1][64:65, :])
                    nc.vector.reciprocal(rsp, rsp)
                    r0 = npool.tile([1, 2048], F16, tag="r0")
                    nc.gpsimd.dma_start(out=r0, in_=rsp)  # SWDGE cast f32->f16
                    bc = npool.tile([64, 2048], F16, tag="bc")
                    nc.gpsimd.partition_broadcast(bc, r0)
                    bcs = [bc[:, bass.ds(1024 * hp, 1024)] for hp in range(2)]
                    for hp in range(2):
                        nc.vector.tensor_tensor(
                            out=aT[hp][0:64, bass.ts(s, 512)],
                            in0=pvs[hp][0:64, 0:512], in1=bcs[hp][:, 0:512],
                            op=OP.mult)
                        nc.vector.tensor_tensor(
                            out=stag[hp][0:64, bass.ts(s, 512)],
                            in0=pvs[hp][0:64, 512:1024],
                            in1=bcs[hp][:, 512:1024], op=OP.mult)
                        nc.sync.dma_start(out=aT[hp][64:128, bass.ts(s, 512)],
                                          in_=stag[hp][0:64, bass.ts(s, 512)])

                    for st in range(4 * s, 4 * s + 4):
                        psos = [ps3.tile([128, 512], F32, tag="pso", name=f"pso{st}_{_d}")
                                for _d in range(2)]
                        for dstrip in range(2):
                            for ch in range(2):
                                nc.tensor.matmul(psos[dstrip], aT[ch][:, bass.ts(st, 128)],
                                                 wo_sb[:, ch, bass.ts(dstrip, 512)],
                                                 start=(ch == 0), stop=(ch == 1))
                        ost = opool.tile([128, 1024], F16, tag="o")
                        if st % 2 == 0:
                            nc.scalar.copy(ost[:, 0:512], psos[0])
                        else:
                            nc.vector.tensor_copy(ost[:, 0:512], psos[0])
                        nc.vector.tensor_copy(ost[:, 512:1024], psos[1])
                        nc.sync.dma_start(out=o_part[bass.ts(st, 128), 0:512],
                                          in_=ost[:, 0:512])
                        nc.gpsimd.dma_start(out=o_part[bass.ts(st, 128), 512:1024],
                                            in_=ost[:, 512:1024])
                # lowest-priority warmkeeper matmuls: run only when the PE has
                # no real work, keeping the HAM p-state from decaying through
                # normalize-chain stalls
                warmkeepers(64)

    nc.compile()
    return nc


_NC = {}


def _get_nc(debug=False):
    if debug not in _NC:
        _NC[debug] = _build()
    return _NC[debug]


def _host_inputs(hidden_states, position_ids, Wq, Wk, Wv, Wo):
    hs = np.asarray(hidden_states, np.float32)
    Wq = np.asarray(Wq, np.float32)
    Wk = np.asarray(Wk, np.float32)
    Wv = np.asarray(Wv, np.float32)
    Wo = np.asarray(Wo, np.float32)

    hsT16 = [np.ascontiguousarray(hs[b].T).astype(np.float16).reshape(8, 128, S)
             for b in range(B)]

    inv_freq = (1.0 / (THETA ** (np.arange(0, HD, 2, dtype=np.float32) / HD))).astype(np.float32)
    cos2, sin2 = [], []
    for b in range(B):
        pos = np.asarray(position_ids[b]).astype(np.float32)
        freqs = pos[:, None] * inv_freq[None, :]          # [S, 32]
        cosf = np.cos(freqs).T                            # [32, S]
        sinf = np.sin(freqs).T
        cos64 = np.concatenate([cosf, cosf], axis=0)      # [64, S]
        sin64s = np.concatenate([-sinf, sinf], axis=0)    # sign-folded
        cos2.append(np.concatenate([cos64, cos64], axis=0).astype(np.float16))
        sin2.append(np.concatenate([sin64s, sin64s], axis=0).astype(np.float16))

    p = np.arange(128)[:, None]
    c = np.arange(WIN)[None, :]
    bandmask = ((p <= c) & (c < p + WINDOW)).astype(np.float16)

    perm = np.zeros((64, 64), np.float32)
    for i in range(32):
        perm[i, i + 32] = 1.0
        perm[i + 32, i] = 1.0
    perm2 = np.kron(np.eye(2, dtype=np.float32), perm)    # [128, 128]
    permT = np.ascontiguousarray(perm2.T).astype(np.float16)

    in_maps = []
    for core in range(8):
        b, g = divmod(core, 4)
        wqT = np.ascontiguousarray(Wq[256 * g:256 * (g + 1)].T).astype(np.float16).reshape(8, 128, 256)
        wkT = np.ascontiguousarray(Wk[64 * g:64 * (g + 1)].T).astype(np.float16).reshape(8, 128, 64)
        wvT = np.ascontiguousarray(Wv[64 * g:64 * (g + 1)].T).astype(np.float16).reshape(8, 128, 64)
        wkvT = np.concatenate([wkT, wvT], axis=2)         # K cols 0:64, V cols 64:128
        woT = np.ascontiguousarray(Wo[:, 256 * g:256 * (g + 1)].T).astype(np.float16).reshape(2, 128, D)
        in_maps.append({
            "hsT": hsT16[b], "wqT": wqT, "wkvT": wkvT, "woT": woT,
            "cos2": cos2[b], "sinS2": sin2[b],
            "bandmask": bandmask, "permT": permT,
        })
    return in_maps


def run_spmd(hidden_states, attention_mask, position_ids, Wq, Wk, Wv, Wo, debug=False, **spmd_kwargs):
    nc = _get_nc(debug)
    in_maps = _host_inputs(hidden_states, position_ids, Wq, Wk, Wv, Wo)
    res = run_bass_kernel_spmd(nc, in_maps, list(range(8)), **spmd_kwargs)
    out = np.zeros((B, S, D), np.float32)
    for core in range(8):
        out[core // 4] += np.asarray(res.results[core]["o_part"], np.float32)
    return out, res


def kernel(hidden_states, attention_mask, position_ids, Wq, Wk, Wv, Wo):
    out, _ = run_spmd(hidden_states, attention_mask, position_ids, Wq, Wk, Wv, Wo)
    return out


# revision 39
# speedup vs baseline: 1.0058x; 1.0058x over previous
"""Mimi-style GQA attention (RoPE + 250-wide sliding causal window) on 8 TRN2 NeuronCores.

Sharding (per spec hint): core c handles batch b=c//4 and KV-head group g=c%4
(4 query heads + 1 KV head). Wq/Wk/Wv column-sharded, Wo row-sharded along the
head dim; each core emits a partial [S, D] fp16 output; host sums the 4
partials per batch in fp32.

Per-core pipeline (all matmul inputs fp16, PSUM accumulation fp32):
  1. qT projections plus a PACKED kv projection (K in PSUM rows 0:64, V in
     rows 64:128) from host-pre-transposed hsT, dt iterated in DMA-arrival
     order across the two load queues. RoPE applied as raw*cos +
     (PERM @ raw)*sin_signed with the PERM matmul on the tensor engine; the
     roped K is partition-shift-DMA'd up to rows 64:128 of kdup so score
     matmuls row-pack two heads; V is transposed per k-tile straight out of
     rows 64:128 (identity block lives at partitions 64:128).
  2. Banded flash attention in scoresT [k, q] orientation: per 128-wide k-tile
     only the 384-wide in-window q-range is computed; exp (with fused 1/8
     scale) on ScalarE; band mask multiply on VectorE over only the two
     136-wide uncertain column bands; PV accumulated per 512-wide q-strip into
     [65, 2, 512] PSUM tiles where row 64 (a ones column in the V stationary)
     accumulates the softmax denominators for free.  No PSUM pre-seeding: the
     diagonal piece clears each bank with start=True and the piece order keeps
     every matmul range uniformly fresh-or-written (kt=4s+1 split at col 384).
  3. PV PSUM is drained to SBUF fp16 immediately (frees the bank for the next
     strip); denominators (row 64) go through one merged per-strip
     reshape-DMA -> f32 reciprocal -> broadcast chain; normalize multiplies
     are cheap fp16 SBUF ops; row-sharded Wo projection emits fp16 partials,
     stores on the SP queue (the tail strip's split across both queues).  Lowest-priority
     "warmkeeper" matmuls run whenever the tensor engine would idle, keeping
     the HAM p-state warm through normalize-chain stalls.
"""
import os
import sys

for _p in ("/opt/trn_rl_repo", "/root/.axon_site/_ro/trn_rl_repo"):
    if os.path.isdir(_p) and _p not in sys.path:
        sys.path.append(_p)

import numpy as np
import concourse.bass as bass
import concourse.mybir as mybir
import concourse.tile as tile
from concourse import bacc
from concourse.bass_utils import run_bass_kernel_spmd
from concourse.masks import make_identity

F32 = mybir.dt.float32
F16 = mybir.dt.float16
AF = mybir.ActivationFunctionType
OP = mybir.AluOpType

B, S, D = 2, 2048, 1024
H, HK, HD = 16, 4, 64
WINDOW = 250
SCALE = 1.0 / np.sqrt(HD)
THETA = 10000.0
NKT = S // 128          # 16 k-tiles
NST = S // 512          # 4 q-strips
WIN = 384               # padded per-k-tile q-window


def _pv_pieces(s):
    """PV pieces for q-strip s: list of (kt, c0, c1) window-column ranges.

    Ordered so the first piece (the diagonal kt=4s, covering strip cols
    0:384 with start=True) clears the PSUM bank, and every later piece's
    column range is uniformly already-written (accumulate) or uniformly
    fresh (overwrite via cleared has_written).  kt=4s+1 is split at the
    384 boundary to keep that invariant.
    """
    ranges = {}
    for kt in range(max(0, 4 * s - 2), min(NKT - 1, 4 * s + 3) + 1):
        j0 = 128 * kt
        w = min(WIN, S - j0)
        c_lo = max(0, 512 * s - j0)
        c_hi = min(w, 512 * (s + 1) - j0)
        if c_lo < c_hi:
            ranges[kt] = (c_lo, c_hi)
    out = []
    for kt in (4 * s, 4 * s + 1, 4 * s + 2, 4 * s + 3, 4 * s - 1, 4 * s - 2):
        if kt not in ranges:
            continue
        c0, c1 = ranges[kt]
        if kt == 4 * s + 1 and c0 < 256 < c1:
            out.append((kt, c0, 256))
            out.append((kt, 256, c1))
        else:
            out.append((kt, c0, c1))
    return out


def _build():
    nc = bacc.Bacc(None, target_bir_lowering=False)

    hsT = nc.declare_dram_parameter("hsT", [8, 128, S], F16, isOutput=False)
    wq = nc.declare_dram_parameter("wqT", [8, 128, 256], F16, isOutput=False)
    wkv = nc.declare_dram_parameter("wkvT", [8, 128, 128], F16, isOutput=False)
    wo = nc.declare_dram_parameter("woT", [2, 128, D], F16, isOutput=False)
    cosd = nc.declare_dram_parameter("cos2", [128, S], F16, isOutput=False)
    sind = nc.declare_dram_parameter("sinS2", [128, S], F16, isOutput=False)
    maskd = nc.declare_dram_parameter("bandmask", [128, WIN], F16, isOutput=False)
    permd = nc.declare_dram_parameter("permT", [128, 128], F16, isOutput=False)
    o_part = nc.declare_dram_parameter("o_part", [S, D], F16, isOutput=True)

    with tile.TileContext(nc) as tc:
        with (
            tc.tile_pool(name="persist", bufs=1) as pp,
            tc.tile_pool(name="work", bufs=3) as wk_pool,
            tc.tile_pool(name="expm", bufs=30) as ep,
            tc.tile_pool(name="norm", bufs=2) as npool,
            tc.tile_pool(name="ost", bufs=4) as opool,
        ):
            # ---- persistent loads, split across the SP and Pool DMA queues,
            # ordered by first use so early matmuls start ASAP ----
            ht = [pp.tile([128, S], F16, tag=f"ht{dt}", name=f"ht{dt}")
                  for dt in range(8)]
            wq_sb = pp.tile([128, 8, 256], F16, tag="wq")
            nc.sync.dma_start(out=wq_sb[:, :, 0:128],
                              in_=wq.rearrange("a p c -> p a c")[:, :, 0:128])
            nc.sync.dma_start(out=ht[0][:, 0:1024], in_=hsT[0][:, 0:1024])
            nc.sync.dma_start(out=ht[0][:, 1024:2048], in_=hsT[0][:, 1024:2048])
            for dt in range(1, 4):
                nc.sync.dma_start(out=ht[dt], in_=hsT[dt])
            wkv_sb = pp.tile([128, 8, 128], F16, tag="wkv")
            nc.gpsimd.dma_start(out=wkv_sb, in_=wkv.rearrange("a p c -> p a c"))
            for dt in range(4, 8):
                nc.gpsimd.dma_start(out=ht[dt], in_=hsT[dt])
            cos_sb = pp.tile([128, S], F16, tag="cos")
            sin_sb = pp.tile([128, S], F16, tag="sin")
            nc.gpsimd.dma_start(out=cos_sb, in_=cosd[:, :])
            nc.gpsimd.dma_start(out=sin_sb, in_=sind[:, :])
            perm_sb = pp.tile([128, 128], F16, tag="perm")
            nc.gpsimd.dma_start(out=perm_sb, in_=permd[:, :])
            mask_sb = pp.tile([128, WIN], F16, tag="mask")
            nc.gpsimd.dma_start(out=mask_sb, in_=maskd[:, :])
            wo_sb = pp.tile([128, 2, D], F16, tag="wo")
            nc.gpsimd.dma_start(out=wo_sb, in_=wo.rearrange("a p c -> p a c"))
            nc.sync.dma_start(out=wq_sb[:, :, 128:256],
                              in_=wq.rearrange("a p c -> p a c")[:, :, 128:256])
            ident = pp.tile([128, 64], F16, tag="ident")
            make_identity(nc, ident[64:128, :])
            wsrc = pp.tile([128, WIN], F16, tag="wsrc")
            nc.gpsimd.memset(wsrc, 0.0)

            qT = [pp.tile([128, S], F16, tag=f"qT{m}", name=f"qT{m}") for m in range(2)]
            kdup = pp.tile([128, S], F16, tag="kdup")
            vhi = pp.tile([128, S], F16, tag="vhi")
            vaug = [pp.tile([128, 65], F16, tag=f"vaug{kt}", name=f"vaug{kt}") for kt in range(NKT)]
            aT = [pp.tile([128, S], F16, tag=f"aT{m}", name=f"aT{m}") for m in range(2)]
            stag = [pp.tile([64, S], F16, tag=f"stag{m}", name=f"stag{m}") for m in range(2)]

            # ---- phase 1: projections + rope ----
            with (
                tc.tile_pool(name="ps1", bufs=6, space="PSUM") as ps1,
                tc.tile_pool(name="ps1b", bufs=2, space="PSUM") as ps1b,
            ):
                def proj_group(lhs_of_dt, consume, gname):
                    """One projection over all 4 strips, dt-outer so each
                    stationary is loaded once and reused across strips."""
                    pstiles = [ps1.tile([128, 512], F32, tag="proj",
                                        name=f"pj{gname}_{_s}") for _s in range(NST)]
                    dts = (0, 4, 1, 5, 2, 6, 3, 7)
                    for i, dt in enumerate(dts):
                        for strip in range(NST):
                            nc.tensor.matmul(pstiles[strip], lhs_of_dt(dt),
                                             ht[dt][:, bass.ts(strip, 512)],
                                             start=(i == 0), stop=(i == 7))
                    for strip in range(NST):
                        consume(pstiles[strip], strip)

                def q_consume(m):
                    def go(ps, strip):
                        sl = bass.ts(strip, 512)
                        raw = wk_pool.tile([128, 512], F16, tag="rawq")
                        nc.scalar.copy(raw, ps)
                        rot = ps1b.tile([128, 512], F32, tag="rot", name="rotq")
                        nc.tensor.matmul(rot, perm_sb, raw, start=True, stop=True)
                        t1 = wk_pool.tile([128, 512], F16, tag="t1q")
                        nc.vector.tensor_tensor(out=t1, in0=rot, in1=sin_sb[:, sl],
                                                op=OP.mult)
                        t2 = wk_pool.tile([128, 512], F16, tag="t2q")
                        nc.gpsimd.tensor_tensor(out=t2, in0=raw, in1=cos_sb[:, sl],
                                                op=OP.mult)
                        nc.vector.tensor_tensor(out=qT[m][:, sl], in0=t1, in1=t2,
                                                op=OP.add)
                    return go

                def kv_consume(ps, strip):
                    sl = bass.ts(strip, 512)
                    # K sits in PSUM rows 0:64 — rope it exactly like baseline
                    kraw = wk_pool.tile([64, 512], F16, tag="kraw")
                    nc.scalar.copy(kraw, ps[0:64, :])
                    rot = ps1b.tile([64, 512], F32, tag="rot", name="rotk")
                    nc.tensor.matmul(rot, perm_sb[0:64, 0:64], kraw,
                                     start=True, stop=True)
                    t1 = wk_pool.tile([64, 512], F16, tag="t1k")
                    nc.vector.tensor_tensor(out=t1, in0=rot, in1=sin_sb[0:64, sl],
                                            op=OP.mult)
                    t2 = wk_pool.tile([64, 512], F16, tag="t2k")
                    nc.gpsimd.tensor_tensor(out=t2, in0=kraw, in1=cos_sb[0:64, sl],
                                            op=OP.mult)
                    nc.vector.tensor_tensor(out=kdup[0:64, sl], in0=t1, in1=t2,
                                            op=OP.add)
                    nc.sync.dma_start(out=kdup[64:128, sl], in_=kdup[0:64, sl])
                    # V sits in rows 64:128; transposes read it there directly
                    nc.vector.tensor_copy(vhi[64:128, sl], ps[64:128, :])

                proj_group(lambda dt: wq_sb[:, dt, bass.ts(0, 128)],
                           q_consume(0), "q0")
                proj_group(lambda dt: wkv_sb[:, dt], kv_consume, "kv")
                proj_group(lambda dt: wq_sb[:, dt, bass.ts(1, 128)],
                           q_consume(1), "q1")

                for kt in range(NKT):
                    pvt = ps1b.tile([128, 64], F16, tag="rot", name=f"vtr{kt}")
                    nc.tensor.transpose(pvt, vhi[64:128, bass.ts(kt, 128)],
                                        ident[64:128, :])
                    nc.vector.tensor_copy(vaug[kt][:, 0:64], pvt)
                    nc.vector.memset(vaug[kt][:, 64:65], 1.0)

            # ---- phases 2+3: banded attention + output projection ----
            with (
                tc.tile_pool(name="ps2", bufs=2, space="PSUM") as ps2,
                tc.tile_pool(name="pspv", bufs=2, space="PSUM") as pspv,
                tc.tile_pool(name="ps3", bufs=2, space="PSUM") as ps3,
            ):
                wk_count = [0]

                def warmkeepers(n):
                    for _ in range(n):
                        i = wk_count[0]
                        wk_count[0] += 1
                        wps = ps2.tile([128, WIN], F32, tag="sc", name=f"warm{i}")
                        nc.tensor.matmul(wps, wsrc[:, 0:128], wsrc,
                                         start=True, stop=True)

                expm = {}
                for s in range(NST):
                    # fresh expT tiles for this strip's new k-tiles
                    kts = range(max(0, 4 * s - 2), min(NKT - 1, 4 * s + 3) + 1)
                    for kt in kts:
                        if (kt, 0) in expm:
                            continue
                        j0 = 128 * kt
                        w = min(WIN, S - j0)
                        for h in range(4):
                            m, half = h // 2, (h % 2) * 64
                            pss = ps2.tile([128, WIN], F32, tag="sc")
                            nc.tensor.matmul(
                                pss[:, 0:w],
                                kdup[half:half + 64, bass.ts(kt, 128)],
                                qT[m][half:half + 64, j0:j0 + w],
                                start=True, stop=True)
                            et = ep.tile([128, WIN], F16, tag="e")
                            nc.scalar.activation(et[:, 0:w], pss[:, 0:w], AF.Exp,
                                                 scale=float(SCALE))
                            if w == WIN:
                                # only cols [0,136) and [248,384) are mask-
                                # uncertain; the middle is always in-band
                                blk = bass.AP(tensor=et.tensor, offset=et.offset,
                                              ap=[list(et.ap[0]), [248, 2], [1, 136]])
                                mblk = bass.AP(tensor=mask_sb.tensor, offset=mask_sb.offset,
                                               ap=[list(mask_sb.ap[0]), [248, 2], [1, 136]])
                                nc.vector.tensor_tensor(out=blk, in0=blk, in1=mblk,
                                                        op=OP.mult)
                            else:
                                nc.vector.tensor_tensor(out=et[:, 0:w], in0=et[:, 0:w],
                                                        in1=mask_sb[:, 0:w], op=OP.mult)
                            expm[(kt, h)] = et

                    pieces = _pv_pieces(s)
                    pvs = []
                    for hp in range(2):
                        pvp = pspv.tile([65, 2, 512], F32, tag="pv",
                                        name=f"pv{hp}_{s}")
                        for i, (kt, c0, c1) in enumerate(pieces):
                            base = 128 * kt + c0 - 512 * s
                            last = i == len(pieces) - 1
                            for hh in range(2):
                                nc.tensor.matmul(pvp[:, hh, base:base + (c1 - c0)],
                                                 vaug[kt],
                                                 expm[(kt, 2 * hp + hh)][:, c0:c1],
                                                 start=(i == 0), stop=last)
                        # drain to SBUF fp16 right away to free the PSUM bank
                        pv16 = npool.tile([65, 1024], F16, tag=f"pv16_{hp}",
                                          name=f"pv16_{hp}_{s}")
                        if hp == 0:
                            nc.scalar.copy(pv16, pvp)
                        else:
                            nc.vector.tensor_copy(pv16, pvp)
                        pvs.append(pv16)

                    # softmax denominators -> broadcast reciprocals.
                    # Early strips: reshape-DMA + f32 reciprocal (cheap on
                    # engines, long latency, hidden under later strips).
                    # Tail strips: recip = exp(-ln(den)) on the [1,1024] rows
                    # (same ACT table set as the score exps) — short latency
                    # chain since nothing hides it at the end of the kernel.
                    rsp = npool.tile([16, 128], F32, tag="rsp")
                    nc.gpsimd.dma_start(out=rsp[0:8, :], in_=pvs[0][64:65, :])
                    nc.gpsimd.dma_start(out=rsp[8:16, :], in_=pvs[1][64:65, :])
                    nc.vector.reciprocal(rsp, rsp)
                    r0 = npool.tile([1, 2048], F16, tag="r0")
                    nc.gpsimd.dma_start(out=r0, in_=rsp)  # SWDGE cast f32->f16
                    bc = npool.tile([64, 2048], F16, tag="bc")
                    nc.gpsimd.partition_broadcast(bc, r0)
                    bcs = [bc[:, bass.ds(1024 * hp, 1024)] for hp in range(2)]
                    for hp in range(2):
                        nc.vector.tensor_tensor(
                            out=aT[hp][0:64, bass.ts(s, 512)],
                            in0=pvs[hp][0:64, 0:512], in1=bcs[hp][:, 0:512],
                            op=OP.mult)
                        nc.vector.tensor_tensor(
                            out=stag[hp][0:64, bass.ts(s, 512)],
                            in0=pvs[hp][0:64, 512:1024],
                            in1=bcs[hp][:, 512:1024], op=OP.mult)
                        nc.sync.dma_start(out=aT[hp][64:128, bass.ts(s, 512)],
                                          in_=stag[hp][0:64, bass.ts(s, 512)])

                    for st in range(4 * s, 4 * s + 4):
                        psos = [ps3.tile([128, 512], F32, tag="pso", name=f"pso{st}_{_d}")
                                for _d in range(2)]
                        for dstrip in range(2):
                            for ch in range(2):
                                nc.tensor.matmul(psos[dstrip], aT[ch][:, bass.ts(st, 128)],
                                                 wo_sb[:, ch, bass.ts(dstrip, 512)],
                                                 start=(ch == 0), stop=(ch == 1))
                        ost = opool.tile([128, 1024], F16, tag="o")
                        if st % 2 == 0:
                            nc.scalar.copy(ost[:, 0:512], psos[0])
                        else:
                            nc.vector.tensor_copy(ost[:, 0:512], psos[0])
                        nc.vector.tensor_copy(ost[:, 512:1024], psos[1])
                        if s == NST - 1:
                            # tail: split across both queues for parallelism
                            nc.sync.dma_start(out=o_part[bass.ts(st, 128), 0:512],
                                              in_=ost[:, 0:512])
                            nc.gpsimd.dma_start(out=o_part[bass.ts(st, 128), 512:1024],
                                                in_=ost[:, 512:1024])
                        else:
                            # keep the Pool queue clear for the next strip's
                            # denominator chain (rsp/r0 DMAs + broadcast)
                            nc.sync.dma_start(out=o_part[bass.ts(st, 128), :],
                                              in_=ost)
                # lowest-priority warmkeeper matmuls: run only when the PE has
                # no real work, keeping the HAM p-state from decaying through
                # normalize-chain stalls
                warmkeepers(64)

    nc.compile()
    return nc


_NC = {}


def _get_nc(debug=False):
    if debug not in _NC:
        _NC[debug] = _build()
    return _NC[debug]


def _host_inputs(hidden_states, position_ids, Wq, Wk, Wv, Wo):
    hs = np.asarray(hidden_states, np.float32)
    Wq = np.asarray(Wq, np.float32)
    Wk = np.asarray(Wk, np.float32)
    Wv = np.asarray(Wv, np.float32)
    Wo = np.asarray(Wo, np.float32)

    hsT16 = [np.ascontiguousarray(hs[b].T).astype(np.float16).reshape(8, 128, S)
             for b in range(B)]

    inv_freq = (1.0 / (THETA ** (np.arange(0, HD, 2, dtype=np.float32) / HD))).astype(np.float32)
    cos2, sin2 = [], []
    for b in range(B):
        pos = np.asarray(position_ids[b]).astype(np.float32)
        freqs = pos[:, None] * inv_freq[None, :]          # [S, 32]
        cosf = np.cos(freqs).T                            # [32, S]
        sinf = np.sin(freqs).T
        cos64 = np.concatenate([cosf, cosf], axis=0)      # [64, S]
        sin64s = np.concatenate([-sinf, sinf], axis=0)    # sign-folded
        cos2.append(np.concatenate([cos64, cos64], axis=0).astype(np.float16))
        sin2.append(np.concatenate([sin64s, sin64s], axis=0).astype(np.float16))

    p = np.arange(128)[:, None]
    c = np.arange(WIN)[None, :]
    bandmask = ((p <= c) & (c < p + WINDOW)).astype(np.float16)

    perm = np.zeros((64, 64), np.float32)
    for i in range(32):
        perm[i, i + 32] = 1.0
        perm[i + 32, i] = 1.0
    perm2 = np.kron(np.eye(2, dtype=np.float32), perm)    # [128, 128]
    permT = np.ascontiguousarray(perm2.T).astype(np.float16)

    in_maps = []
    for core in range(8):
        b, g = divmod(core, 4)
        wqT = np.ascontiguousarray(Wq[256 * g:256 * (g + 1)].T).astype(np.float16).reshape(8, 128, 256)
        wkT = np.ascontiguousarray(Wk[64 * g:64 * (g + 1)].T).astype(np.float16).reshape(8, 128, 64)
        wvT = np.ascontiguousarray(Wv[64 * g:64 * (g + 1)].T).astype(np.float16).reshape(8, 128, 64)
        wkvT = np.concatenate([wkT, wvT], axis=2)         # K cols 0:64, V cols 64:128
        woT = np.ascontiguousarray(Wo[:, 256 * g:256 * (g + 1)].T).astype(np.float16).reshape(2, 128, D)
        in_maps.append({
            "hsT": hsT16[b], "wqT": wqT, "wkvT": wkvT, "woT": woT,
            "cos2": cos2[b], "sinS2": sin2[b],
            "bandmask": bandmask, "permT": permT,
        })
    return in_maps


def run_spmd(hidden_states, attention_mask, position_ids, Wq, Wk, Wv, Wo, debug=False, **spmd_kwargs):
    nc = _get_nc(debug)
    in_maps = _host_inputs(hidden_states, position_ids, Wq, Wk, Wv, Wo)
    res = run_bass_kernel_spmd(nc, in_maps, list(range(8)), **spmd_kwargs)
    out = np.zeros((B, S, D), np.float32)
    for core in range(8):
        out[core // 4] += np.asarray(res.results[core]["o_part"], np.float32)
    return out, res


def kernel(hidden_states, attention_mask, position_ids, Wq, Wk, Wv, Wo):
    out, _ = run_spmd(hidden_states, attention_mask, position_ids, Wq, Wk, Wv, Wo)
    return out


# revision 42
# speedup vs baseline: 1.0066x; 1.0008x over previous
"""Mimi-style GQA attention (RoPE + 250-wide sliding causal window) on 8 TRN2 NeuronCores.

Sharding (per spec hint): core c handles batch b=c//4 and KV-head group g=c%4
(4 query heads + 1 KV head). Wq/Wk/Wv column-sharded, Wo row-sharded along the
head dim; each core emits a partial [S, D] fp16 output; host sums the 4
partials per batch in fp32.

Per-core pipeline (all matmul inputs fp16, PSUM accumulation fp32):
  1. qT projections plus a PACKED kv projection (K in PSUM rows 0:64, V in
     rows 64:128) from host-pre-transposed hsT, dt iterated in DMA-arrival
     order across the two load queues. RoPE applied as raw*cos +
     (PERM @ raw)*sin_signed with the PERM matmul on the tensor engine; the
     roped K is partition-shift-DMA'd up to rows 64:128 of kdup so score
     matmuls row-pack two heads; V is transposed per k-tile straight out of
     rows 64:128 (identity block lives at partitions 64:128).
  2. Banded flash attention in scoresT [k, q] orientation: per 128-wide k-tile
     only the 384-wide in-window q-range is computed; exp (with fused 1/8
     scale) on ScalarE; band mask multiply on VectorE over only the two
     136-wide uncertain column bands; PV accumulated per 512-wide q-strip into
     [65, 2, 512] PSUM tiles where row 64 (a ones column in the V stationary)
     accumulates the softmax denominators for free.  No PSUM pre-seeding: the
     diagonal piece clears each bank with start=True and the piece order keeps
     every matmul range uniformly fresh-or-written (kt=4s+1 split at col 384).
  3. PV PSUM is drained to SBUF fp16 immediately (frees the bank for the next
     strip); denominators (row 64) go through one merged per-strip
     reshape-DMA -> f32 reciprocal -> broadcast chain; normalize multiplies
     are cheap fp16 SBUF ops; row-sharded Wo projection emits fp16 partials,
     stores on the SP queue (the tail strip's split across both queues).  Lowest-priority
     "warmkeeper" matmuls run whenever the tensor engine would idle, keeping
     the HAM p-state warm through normalize-chain stalls.
"""
import os
import sys

for _p in ("/opt/trn_rl_repo", "/root/.axon_site/_ro/trn_rl_repo"):
    if os.path.isdir(_p) and _p not in sys.path:
        sys.path.append(_p)

import numpy as np
import concourse.bass as bass
import concourse.mybir as mybir
import concourse.tile as tile
from concourse import bacc
from concourse.bass_utils import run_bass_kernel_spmd
from concourse.masks import make_identity

F32 = mybir.dt.float32
F16 = mybir.dt.float16
AF = mybir.ActivationFunctionType
OP = mybir.AluOpType

B, S, D = 2, 2048, 1024
H, HK, HD = 16, 4, 64
WINDOW = 250
SCALE = 1.0 / np.sqrt(HD)
THETA = 10000.0
NKT = S // 128          # 16 k-tiles
NST = S // 512          # 4 q-strips
WIN = 384               # padded per-k-tile q-window


def _pv_pieces(s):
    """PV pieces for q-strip s: list of (kt, c0, c1) window-column ranges.

    Ordered so the first piece (the diagonal kt=4s, covering strip cols
    0:384 with start=True) clears the PSUM bank, and every later piece's
    column range is uniformly already-written (accumulate) or uniformly
    fresh (overwrite via cleared has_written).  kt=4s+1 is split at the
    384 boundary to keep that invariant.
    """
    ranges = {}
    for kt in range(max(0, 4 * s - 2), min(NKT - 1, 4 * s + 3) + 1):
        j0 = 128 * kt
        w = min(WIN, S - j0)
        c_lo = max(0, 512 * s - j0)
        c_hi = min(w, 512 * (s + 1) - j0)
        if c_lo < c_hi:
            ranges[kt] = (c_lo, c_hi)
    out = []
    for kt in (4 * s, 4 * s + 1, 4 * s + 2, 4 * s + 3, 4 * s - 1, 4 * s - 2):
        if kt not in ranges:
            continue
        c0, c1 = ranges[kt]
        if kt == 4 * s + 1 and c0 < 256 < c1:
            out.append((kt, c0, 256))
            out.append((kt, 256, c1))
        else:
            out.append((kt, c0, c1))
    return out


def _build():
    nc = bacc.Bacc(None, target_bir_lowering=False)

    hsT = nc.declare_dram_parameter("hsT", [8, 128, S], F16, isOutput=False)
    wq = nc.declare_dram_parameter("wqT", [8, 128, 256], F16, isOutput=False)
    wkv = nc.declare_dram_parameter("wkvT", [8, 128, 128], F16, isOutput=False)
    wo = nc.declare_dram_parameter("woT", [2, 128, D], F16, isOutput=False)
    cosd = nc.declare_dram_parameter("cos2", [128, S], F16, isOutput=False)
    sind = nc.declare_dram_parameter("sinS2", [128, S], F16, isOutput=False)
    maskd = nc.declare_dram_parameter("bandmask", [128, WIN], F16, isOutput=False)
    permd = nc.declare_dram_parameter("permT", [128, 128], F16, isOutput=False)
    o_part = nc.declare_dram_parameter("o_part", [S, D], F16, isOutput=True)

    with tile.TileContext(nc) as tc:
        with (
            tc.tile_pool(name="persist", bufs=1) as pp,
            tc.tile_pool(name="work", bufs=3) as wk_pool,
            tc.tile_pool(name="expm", bufs=30) as ep,
            tc.tile_pool(name="norm", bufs=2) as npool,
            tc.tile_pool(name="ost", bufs=4) as opool,
        ):
            # ---- persistent loads, split across the SP and Pool DMA queues,
            # ordered by first use so early matmuls start ASAP ----
            ht = [pp.tile([128, S], F16, tag=f"ht{dt}", name=f"ht{dt}")
                  for dt in range(8)]
            wq_sb = pp.tile([128, 8, 256], F16, tag="wq")
            nc.sync.dma_start(out=wq_sb[:, :, 0:128],
                              in_=wq.rearrange("a p c -> p a c")[:, :, 0:128])
            nc.sync.dma_start(out=ht[0][:, 0:1024], in_=hsT[0][:, 0:1024])
            nc.sync.dma_start(out=ht[0][:, 1024:2048], in_=hsT[0][:, 1024:2048])
            for dt in range(1, 4):
                nc.sync.dma_start(out=ht[dt], in_=hsT[dt])
            wkv_sb = pp.tile([128, 8, 128], F16, tag="wkv")
            nc.gpsimd.dma_start(out=wkv_sb, in_=wkv.rearrange("a p c -> p a c"))
            for dt in range(4, 8):
                nc.gpsimd.dma_start(out=ht[dt], in_=hsT[dt])
            cos_sb = pp.tile([128, S], F16, tag="cos")
            sin_sb = pp.tile([128, S], F16, tag="sin")
            nc.gpsimd.dma_start(out=cos_sb, in_=cosd[:, :])
            nc.gpsimd.dma_start(out=sin_sb, in_=sind[:, :])
            perm_sb = pp.tile([128, 128], F16, tag="perm")
            nc.gpsimd.dma_start(out=perm_sb, in_=permd[:, :])
            mask_sb = pp.tile([128, WIN], F16, tag="mask")
            nc.gpsimd.dma_start(out=mask_sb, in_=maskd[:, :])
            wo_sb = pp.tile([128, 2, D], F16, tag="wo")
            nc.gpsimd.dma_start(out=wo_sb, in_=wo.rearrange("a p c -> p a c"))
            nc.sync.dma_start(out=wq_sb[:, :, 128:256],
                              in_=wq.rearrange("a p c -> p a c")[:, :, 128:256])
            ident = pp.tile([128, 64], F16, tag="ident")
            make_identity(nc, ident[64:128, :])
            wsrc = pp.tile([128, WIN], F16, tag="wsrc")
            nc.gpsimd.memset(wsrc, 0.0)

            qT = [pp.tile([128, S], F16, tag=f"qT{m}", name=f"qT{m}") for m in range(2)]
            kdup = pp.tile([128, S], F16, tag="kdup")
            vhi = pp.tile([128, S], F16, tag="vhi")
            vaug = [pp.tile([128, 65], F16, tag=f"vaug{kt}", name=f"vaug{kt}") for kt in range(NKT)]
            aT = [pp.tile([128, S], F16, tag=f"aT{m}", name=f"aT{m}") for m in range(2)]
            stag = [pp.tile([64, S], F16, tag=f"stag{m}", name=f"stag{m}") for m in range(2)]

            # ---- phase 1: projections + rope ----
            with (
                tc.tile_pool(name="ps1", bufs=6, space="PSUM") as ps1,
                tc.tile_pool(name="ps1b", bufs=2, space="PSUM") as ps1b,
            ):
                def proj_group(lhs_of_dt, consume, gname):
                    """One projection over all 4 strips, dt-outer so each
                    stationary is loaded once and reused across strips."""
                    pstiles = [ps1.tile([128, 512], F32, tag="proj",
                                        name=f"pj{gname}_{_s}") for _s in range(NST)]
                    dts = (0, 4, 1, 5, 2, 6, 3, 7)
                    for i, dt in enumerate(dts):
                        for strip in range(NST):
                            nc.tensor.matmul(pstiles[strip], lhs_of_dt(dt),
                                             ht[dt][:, bass.ts(strip, 512)],
                                             start=(i == 0), stop=(i == 7))
                    for strip in range(NST):
                        consume(pstiles[strip], strip)

                def q_consume(m):
                    def go(ps, strip):
                        sl = bass.ts(strip, 512)
                        raw = wk_pool.tile([128, 512], F16, tag="rawq")
                        nc.scalar.copy(raw, ps)
                        rot = ps1b.tile([128, 512], F32, tag="rot", name="rotq")
                        nc.tensor.matmul(rot, perm_sb, raw, start=True, stop=True)
                        t1 = wk_pool.tile([128, 512], F16, tag="t1q")
                        nc.vector.tensor_tensor(out=t1, in0=rot, in1=sin_sb[:, sl],
                                                op=OP.mult)
                        t2 = wk_pool.tile([128, 512], F16, tag="t2q")
                        nc.gpsimd.tensor_tensor(out=t2, in0=raw, in1=cos_sb[:, sl],
                                                op=OP.mult)
                        nc.vector.tensor_tensor(out=qT[m][:, sl], in0=t1, in1=t2,
                                                op=OP.add)
                    return go

                def kv_consume(ps, strip):
                    sl = bass.ts(strip, 512)
                    # K sits in PSUM rows 0:64 — rope it exactly like baseline
                    kraw = wk_pool.tile([64, 512], F16, tag="kraw")
                    nc.scalar.copy(kraw, ps[0:64, :])
                    rot = ps1b.tile([64, 512], F32, tag="rot", name="rotk")
                    nc.tensor.matmul(rot, perm_sb[0:64, 0:64], kraw,
                                     start=True, stop=True)
                    t1 = wk_pool.tile([64, 512], F16, tag="t1k")
                    nc.vector.tensor_tensor(out=t1, in0=rot, in1=sin_sb[0:64, sl],
                                            op=OP.mult)
                    t2 = wk_pool.tile([64, 512], F16, tag="t2k")
                    nc.gpsimd.tensor_tensor(out=t2, in0=kraw, in1=cos_sb[0:64, sl],
                                            op=OP.mult)
                    nc.vector.tensor_tensor(out=kdup[0:64, sl], in0=t1, in1=t2,
                                            op=OP.add)
                    nc.sync.dma_start(out=kdup[64:128, sl], in_=kdup[0:64, sl])
                    # V sits in rows 64:128; transposes read it there directly
                    nc.vector.tensor_copy(vhi[64:128, sl], ps[64:128, :])

                proj_group(lambda dt: wq_sb[:, dt, bass.ts(0, 128)],
                           q_consume(0), "q0")
                proj_group(lambda dt: wkv_sb[:, dt], kv_consume, "kv")
                proj_group(lambda dt: wq_sb[:, dt, bass.ts(1, 128)],
                           q_consume(1), "q1")

                for kt in range(NKT):
                    pvt = ps1b.tile([128, 64], F16, tag="rot", name=f"vtr{kt}")
                    nc.tensor.transpose(pvt, vhi[64:128, bass.ts(kt, 128)],
                                        ident[64:128, :])
                    nc.vector.tensor_copy(vaug[kt][:, 0:64], pvt)
                    nc.vector.memset(vaug[kt][:, 64:65], 1.0)

            # ---- phases 2+3: banded attention + output projection ----
            with (
                tc.tile_pool(name="ps2", bufs=2, space="PSUM") as ps2,
                tc.tile_pool(name="pspv", bufs=2, space="PSUM") as pspv,
                tc.tile_pool(name="ps3", bufs=2, space="PSUM") as ps3,
            ):
                wk_count = [0]

                def warmkeepers(n):
                    for _ in range(n):
                        i = wk_count[0]
                        wk_count[0] += 1
                        wps = ps2.tile([128, WIN], F32, tag="sc", name=f"warm{i}")
                        nc.tensor.matmul(wps, wsrc[:, 0:128], wsrc,
                                         start=True, stop=True)

                expm = {}
                for s in range(NST):
                    # fresh expT tiles for this strip's new k-tiles
                    kts = range(max(0, 4 * s - 2), min(NKT - 1, 4 * s + 3) + 1)
                    for kt in kts:
                        if (kt, 0) in expm:
                            continue
                        j0 = 128 * kt
                        w = min(WIN, S - j0)
                        for h in range(4):
                            m, half = h // 2, (h % 2) * 64
                            pss = ps2.tile([128, WIN], F32, tag="sc")
                            nc.tensor.matmul(
                                pss[:, 0:w],
                                kdup[half:half + 64, bass.ts(kt, 128)],
                                qT[m][half:half + 64, j0:j0 + w],
                                start=True, stop=True)
                            et = ep.tile([128, WIN], F16, tag="e")
                            nc.scalar.activation(et[:, 0:w], pss[:, 0:w], AF.Exp,
                                                 scale=float(SCALE))
                            if w == WIN:
                                # only cols [0,136) and [248,384) are mask-
                                # uncertain; the middle is always in-band
                                blk = bass.AP(tensor=et.tensor, offset=et.offset,
                                              ap=[list(et.ap[0]), [248, 2], [1, 136]])
                                mblk = bass.AP(tensor=mask_sb.tensor, offset=mask_sb.offset,
                                               ap=[list(mask_sb.ap[0]), [248, 2], [1, 136]])
                                nc.vector.tensor_tensor(out=blk, in0=blk, in1=mblk,
                                                        op=OP.mult)
                            else:
                                nc.vector.tensor_tensor(out=et[:, 0:w], in0=et[:, 0:w],
                                                        in1=mask_sb[:, 0:w], op=OP.mult)
                            expm[(kt, h)] = et

                    pieces = _pv_pieces(s)
                    pvs = []
                    for hp in range(2):
                        pvp = pspv.tile([65, 2, 512], F32, tag="pv",
                                        name=f"pv{hp}_{s}")
                        for i, (kt, c0, c1) in enumerate(pieces):
                            base = 128 * kt + c0 - 512 * s
                            last = i == len(pieces) - 1
                            for hh in range(2):
                                nc.tensor.matmul(pvp[:, hh, base:base + (c1 - c0)],
                                                 vaug[kt],
                                                 expm[(kt, 2 * hp + hh)][:, c0:c1],
                                                 start=(i == 0), stop=last)
                        # drain to SBUF fp16 right away to free the PSUM bank
                        pv16 = npool.tile([65, 1024], F16, tag=f"pv16_{hp}",
                                          name=f"pv16_{hp}_{s}")
                        if hp == 0:
                            nc.scalar.copy(pv16, pvp)
                        else:
                            nc.vector.tensor_copy(pv16, pvp)
                        pvs.append(pv16)

                    # softmax denominators -> broadcast reciprocals.
                    # Early strips: reshape-DMA + f32 reciprocal (cheap on
                    # engines, long latency, hidden under later strips).
                    # Tail strips: recip = exp(-ln(den)) on the [1,1024] rows
                    # (same ACT table set as the score exps) — short latency
                    # chain since nothing hides it at the end of the kernel.
                    rsp = npool.tile([16, 128], F32, tag="rsp")
                    nc.gpsimd.dma_start(out=rsp[0:8, :], in_=pvs[0][64:65, :])
                    nc.gpsimd.dma_start(out=rsp[8:16, :], in_=pvs[1][64:65, :])
                    nc.vector.reciprocal(rsp, rsp)
                    r0 = npool.tile([1, 2048], F16, tag="r0")
                    nc.gpsimd.dma_start(out=r0, in_=rsp)  # SWDGE cast f32->f16
                    bc = npool.tile([64, 2048], F16, tag="bc")
                    # split broadcast per head-pair: hp0's normalize mults can
                    # start while hp1's half is still broadcasting
                    nc.gpsimd.partition_broadcast(bc[:, 0:1024], r0[0:1, 0:1024])
                    nc.gpsimd.partition_broadcast(bc[:, 1024:2048], r0[0:1, 1024:2048])
                    bcs = [bc[:, bass.ds(1024 * hp, 1024)] for hp in range(2)]
                    for hp in range(2):
                        nc.vector.tensor_tensor(
                            out=aT[hp][0:64, bass.ts(s, 512)],
                            in0=pvs[hp][0:64, 0:512], in1=bcs[hp][:, 0:512],
                            op=OP.mult)
                        nc.vector.tensor_tensor(
                            out=stag[hp][0:64, bass.ts(s, 512)],
                            in0=pvs[hp][0:64, 512:1024],
                            in1=bcs[hp][:, 512:1024], op=OP.mult)
                        nc.sync.dma_start(out=aT[hp][64:128, bass.ts(s, 512)],
                                          in_=stag[hp][0:64, bass.ts(s, 512)])

                    for st in range(4 * s, 4 * s + 4):
                        psos = [ps3.tile([128, 512], F32, tag="pso", name=f"pso{st}_{_d}")
                                for _d in range(2)]
                        for dstrip in range(2):
                            for ch in range(2):
                                nc.tensor.matmul(psos[dstrip], aT[ch][:, bass.ts(st, 128)],
                                                 wo_sb[:, ch, bass.ts(dstrip, 512)],
                                                 start=(ch == 0), stop=(ch == 1))
                        ost = opool.tile([128, 1024], F16, tag="o")
                        if st % 2 == 0:
                            nc.scalar.copy(ost[:, 0:512], psos[0])
                        else:
                            nc.vector.tensor_copy(ost[:, 0:512], psos[0])
                        nc.vector.tensor_copy(ost[:, 512:1024], psos[1])
                        if s == NST - 1:
                            # tail: split across both queues for parallelism
                            nc.sync.dma_start(out=o_part[bass.ts(st, 128), 0:512],
                                              in_=ost[:, 0:512])
                            nc.gpsimd.dma_start(out=o_part[bass.ts(st, 128), 512:1024],
                                                in_=ost[:, 512:1024])
                        else:
                            # keep the Pool queue clear for the next strip's
                            # denominator chain (rsp/r0 DMAs + broadcast)
                            nc.sync.dma_start(out=o_part[bass.ts(st, 128), :],
                                              in_=ost)
                # lowest-priority warmkeeper matmuls: run only when the PE has
                # no real work, keeping the HAM p-state from decaying through
                # normalize-chain stalls
                warmkeepers(64)

    nc.compile()
    return nc


_NC = {}


def _get_nc(debug=False):
    if debug not in _NC:
        _NC[debug] = _build()
    return _NC[debug]


def _host_inputs(hidden_states, position_ids, Wq, Wk, Wv, Wo):
    hs = np.asarray(hidden_states, np.float32)
    Wq = np.asarray(Wq, np.float32)
    Wk = np.asarray(Wk, np.float32)
    Wv = np.asarray(Wv, np.float32)
    Wo = np.asarray(Wo, np.float32)

    hsT16 = [np.ascontiguousarray(hs[b].T).astype(np.float16).reshape(8, 128, S)
             for b in range(B)]

    inv_freq = (1.0 / (THETA ** (np.arange(0, HD, 2, dtype=np.float32) / HD))).astype(np.float32)
    cos2, sin2 = [], []
    for b in range(B):
        pos = np.asarray(position_ids[b]).astype(np.float32)
        freqs = pos[:, None] * inv_freq[None, :]          # [S, 32]
        cosf = np.cos(freqs).T                            # [32, S]
        sinf = np.sin(freqs).T
        cos64 = np.concatenate([cosf, cosf], axis=0)      # [64, S]
        sin64s = np.concatenate([-sinf, sinf], axis=0)    # sign-folded
        cos2.append(np.concatenate([cos64, cos64], axis=0).astype(np.float16))
        sin2.append(np.concatenate([sin64s, sin64s], axis=0).astype(np.float16))

    p = np.arange(128)[:, None]
    c = np.arange(WIN)[None, :]
    bandmask = ((p <= c) & (c < p + WINDOW)).astype(np.float16)

    perm = np.zeros((64, 64), np.float32)
    for i in range(32):
        perm[i, i + 32] = 1.0
        perm[i + 32, i] = 1.0
    perm2 = np.kron(np.eye(2, dtype=np.float32), perm)    # [128, 128]
    permT = np.ascontiguousarray(perm2.T).astype(np.float16)

    in_maps = []
    for core in range(8):
        b, g = divmod(core, 4)
        wqT = np.ascontiguousarray(Wq[256 * g:256 * (g + 1)].T).astype(np.float16).reshape(8, 128, 256)
        wkT = np.ascontiguousarray(Wk[64 * g:64 * (g + 1)].T).astype(np.float16).reshape(8, 128, 64)
        wvT = np.ascontiguousarray(Wv[64 * g:64 * (g + 1)].T).astype(np.float16).reshape(8, 128, 64)
        wkvT = np.concatenate([wkT, wvT], axis=2)         # K cols 0:64, V cols 64:128
        woT = np.ascontiguousarray(Wo[:, 256 * g:256 * (g + 1)].T).astype(np.float16).reshape(2, 128, D)
        in_maps.append({
            "hsT": hsT16[b], "wqT": wqT, "wkvT": wkvT, "woT": woT,
            "cos2": cos2[b], "sinS2": sin2[b],
            "bandmask": bandmask, "permT": permT,
        })
    return in_maps


def run_spmd(hidden_states, attention_mask, position_ids, Wq, Wk, Wv, Wo, debug=False, **spmd_kwargs):
    nc = _get_nc(debug)
    in_maps = _host_inputs(hidden_states, position_ids, Wq, Wk, Wv, Wo)
    res = run_bass_kernel_spmd(nc, in_maps, list(range(8)), **spmd_kwargs)
    out = np.zeros((B, S, D), np.float32)
    for core in range(8):
        out[core // 4] += np.asarray(res.results[core]["o_part"], np.float32)
    return out, res


def kernel(hidden_states, attention_mask, position_ids, Wq, Wk, Wv, Wo):
    out, _ = run_spmd(hidden_states, attention_mask, position_ids, Wq, Wk, Wv, Wo)
    return out
